# revision 51
# baseline (speedup 1.0000x reference)
"""LoRALinear fused kernel for 8 trn2 NeuronCores.

y = x @ (base + 2*(B@A))^T + bias,  x:[2,2048,4096], base:[4096,4096],
A:[8,4096], B:[4096,8], bias:[4096] -> y:[2,2048,4096], all fp32.

Sharding: 8 token-shards (pure data-parallel, weights replicated).
Per core:
  y_c[512, 4096] = x_c[512,4096] @ W[4096,4096]^T + bias
decomposed as
  y_c = x_c@base^T + [x_c@A^T | 1] @ [2*B^T ; bias].

Token-sharding (vs. tensor-parallel) means each core owns unique
tokens, so the rank-8 PT = A@x^T is computed exactly once per token —
half the PT work of a 4x2 token-x-dout sharding.  Weights stream
just-in-time (33.6MB/core at ~150GB/s sustained, far under the HBM
limit), x^T stays SBUF-resident (4.2MB).

Main/close matmuls run fp16 (11-bit mantissa, same precision class as
the f32r cast path; PSUM accumulates fp32): the PE matmul stream is
the only roofline (~232us busy/core; ~216ns per 512-wide matmul = the
NX issue-rate bound).  PT runs fp8e4m3 DoubleRow (two k-chunks per
instruction), which only perturbs the LoRA term (~11% of y) -> rel
err ~4.3e-3 against the 2e-2 gate.

Schedule notes (each worth real ns on HW):
- All o-blocks are k-outer (consume weight chunks as they arrive);
  accumulator PSUM tags alternate banks 0-3 / 4-7 between o-blocks so
  a block's first matmuls never wait on the previous block's
  evacuation.  The last o-block is t-outer (its weights are long
  resident) so each token tile drains while the next computes and the
  final-drain tail is a single tile.
- All operand layouts are host-packed so every DMA line is >=2KB
  contiguous per partition (keeps HWDGE descriptor counts at 128).
- Every matmul is [K<=128 x 128]x[K x 512]: A^T/ptw/bb are zero-padded
  to full 128 rows/cols on device because switching the PE's active
  row/col-group mask costs ~100ns each way.
- WARMUP dummy matmuls on a zeroed tile run during the initial DMA
  latency window to flip the HAM clock-gate (1.2 -> 2.4GHz) before
  real work starts.
- fp8 copies of x^T are cast on the DVE as chunks arrive (no extra
  HBM traffic).
"""
import sys

sys.path.insert(0, "/opt/trn_rl_repo")

import numpy as np

T_SH = 8                   # token shards (weights replicated)
T, D, O = 4096, 4096, 4096  # flattened tokens, d_in, d_out
TC, OC = T // T_SH, O       # 512 tokens, 4096 outs per core
KC = D // 128              # 32 contraction chunks
NB = OC // 512             # 8 o-blocks of 512 per core
TT = TC // 128             # 4 token tiles per core

# DMA group sizes (in k-chunks) for the weight stream; first o-block
# starts small so the first matmuls' data lands fast.
WG0 = [1, 1, 2, 4, 4, 4, 8, 8]
WGN = [16, 16]
# xt chunk DMA groups (chunk = [128, TC] fp16 = 128KB)
XG = [1, 1, 2, 4, 4, 4, 8, 8]
# Warm-up matmuls run at the cold 1.2GHz clock (~426ns) and flip the HAM
# clock-gate to 2.4GHz after ~3.4us; sized to end right as the first real
# operands land so they don't delay real work.
WARMUP = 24

_cache = {}


def _build():
    import concourse.mybir as mybir
    import concourse.tile as tile
    from concourse import bacc

    f32 = mybir.dt.float32
    f16 = mybir.dt.float16
    f8 = mybir.dt.float8e4
    DR = mybir.MatmulPerfMode.DoubleRow

    nc = bacc.Bacc("TRN2", target_bir_lowering=False, debug=False,
                   num_devices=8)

    # host-packed: xt[p, k, t] = x^T[k*128+p, t] so DMA lines are contiguous
    xt_d = nc.dram_tensor("xt", [128, KC, TC], f16, kind="ExternalInput").ap()
    # weight stream, host-packed so every DMA line is contiguous:
    # wt[p, ob, k, o] = W^T[k*128+p, ob*512+o]
    wt_d = nc.dram_tensor("wt", [128, NB, KC, 512], f16,
                          kind="ExternalInput").ap()
    at_d = nc.dram_tensor("at", [128, KC // 2, 2, 8], f16,
                          kind="ExternalInput").ap()
    # rows 0-7: 2*B^T, row 8: bias  (K=9 close matmul adds lora + bias)
    bb_d = nc.dram_tensor("bb", [9, OC], f16, kind="ExternalInput").ap()
    ones_d = nc.dram_tensor("ones", [1, TC], f16, kind="ExternalInput").ap()
    y_d = nc.dram_tensor("y", [TC, OC], f32, kind="ExternalOutput").ap()

    with tile.TileContext(nc) as tc:
        with (
            tc.tile_pool(name="res", bufs=1) as res,
            tc.tile_pool(name="wres", bufs=2) as wres,
            tc.tile_pool(name="evac", bufs=8) as evac,
            tc.tile_pool(name="x8", bufs=6) as x8p,
            tc.tile_pool(name="psum", bufs=1, space="PSUM") as psum,
        ):
            # PE warm-up: matmuls on a zeroed SBUF tile (no DMA deps) run
            # while the first loads are in flight, so the HAM clock-gate
            # reaches 2.4GHz before real work starts.
            junk = res.tile([128, 512], f16)
            nc.vector.memset(junk[:], 0)
            wacc = psum.tile([128, 512], f32, name="warm", tag="acc4")
            for _ in range(WARMUP):
                nc.tensor.matmul(wacc[:], junk[:, 0:128], junk[:],
                                 start=True, stop=True)
            # A^T zero-padded to 128 columns, fp8, DoubleRow pair layout
            # [p, k-pair, 2, 128]: PT matmuls contract two k-chunks per
            # instruction at half cost, with 128-wide outputs like everything
            # else (switching the PE column mask costs ~100ns each way).
            atp8 = res.tile([128, KC // 2, 2, 128], f8)
            nc.vector.memset(atp8[:], 0)
            # ptw/bb zero-padded to 128 contraction rows for the same reason:
            # a K=9 close matmul flips the row-group mask (+~100ns twice).
            # bb's big memset rides the idle GpSimd so the DVE reaches the
            # fp8 casts in time for the first PT matmul.
            ptw = res.tile([128, TC], f16)
            bb = res.tile([128, OC], f16)
            nc.gpsimd.memset(bb[:], 0)

            # critical-path loads first: at on sync (ahead of weights),
            # xt chunk 0 on scalar, so the first matmuls start ~10us
            at = res.tile([128, KC // 2, 2, 8], f16)
            nc.sync.dma_start(at[:], at_d[:])
            nc.vector.tensor_copy(atp8[:, :, :, 0:8], at[:])
            nc.vector.memset(ptw[:], 0)
            xt = res.tile([128, KC, TC], f16)
            c0 = 0
            for ng in XG:
                nc.scalar.dma_start(xt[:, c0:c0 + ng, :],
                                    xt_d[:, c0:c0 + ng, :])
                c0 += ng
            # close-time tensors ride behind the full xt stream (first
            # close is at ~40us; these only need to land by then)
            nc.scalar.dma_start(bb[0:9, :], bb_d[:])
            # ptw rows 0-7: PT = A@x^T (device), row 8: ones
            nc.scalar.dma_start(ptw[8:9, :], ones_d[:])

            # fp8 copies of the xt chunks for the DoubleRow PT matmuls, cast
            # on the DVE as each chunk arrives (no extra HBM traffic); the
            # rotating pair-buffers pace themselves via WAR deps
            x8tiles = []
            for j in range(KC // 2):
                x8t = x8p.tile([128, 2, TC], f8, name=f"x8_{j}", tag="x8")
                for i in range(2):
                    nc.vector.tensor_copy(x8t[:, i, :], xt[:, 2 * j + i, :])
                x8tiles.append(x8t)

            def wt_fetch(ob, groups):
                w = wres.tile([128, KC, 512], f16, name=f"wtob{ob}",
                              tag="wtob")
                c0 = 0
                for ng in groups:
                    nc.sync.dma_start(w[:, c0:c0 + ng, :],
                                      wt_d[:, ob, c0:c0 + ng, :])
                    c0 += ng
                return w

            def close_and_evac(acc, t, osl, ring=None, split_out=False,
                               final=False):
                ev = evac.tile([128, 512], f32, name=f"ev{t}", tag="ev")
                tsl = slice(128 * t, 128 * (t + 1))
                nc.tensor.matmul(acc[:], ptw[0:128, 128 * t:128 * (t + 1)],
                                 bb[0:128, osl], start=False, stop=True)
                if final:
                    # very last tile: halve the copy across two engines and
                    # the store across both rings to shorten the drain
                    nc.vector.tensor_copy(ev[:, 0:256], acc[:, 0:256])
                    nc.scalar.copy(ev[:, 256:512], acc[:, 256:512])
                else:
                    nc.vector.tensor_copy(ev[:], acc[:])
                if final or split_out:
                    h = slice(osl.start, osl.start + 256)
                    h2 = slice(osl.start + 256, osl.stop)
                    nc.scalar.dma_start(y_d[tsl, h], ev[:, 0:256])
                    nc.sync.dma_start(y_d[tsl, h2], ev[:, 256:512])
                else:
                    (ring or nc.scalar).dma_start(y_d[tsl, osl], ev[:])

            def o_block(ob, w):
                # k-outer: consumes each weight chunk as it arrives; PSUM
                # accumulator banks alternate 0-3/4-7 between o-blocks.
                # o-block 0 interleaves PT = A_pad@x^T (one extra bank).
                osl = slice(512 * ob, 512 * (ob + 1))
                base = 0 if ob % 2 == 0 else 4
                accs = {
                    t: psum.tile([128, 512], f32, name=f"acc{t}_{ob}",
                                 tag=f"acc{base + t}")
                    for t in range(TT)
                }
                if ob == 0:
                    ptp = psum.tile([128, 512], f32, name="ptp", tag="acc4")
                for k in range(KC):
                    for t in range(TT):
                        nc.tensor.matmul(
                            accs[t][:],
                            xt[:, k, 128 * t:128 * (t + 1)],
                            w[:, k, :],
                            start=(k == 0), stop=False)
                    if ob == 0 and k % 2 == 1:
                        j = k // 2
                        nc.tensor.matmul(ptp[:], atp8[:, j, :, :],
                                         x8tiles[j][:],
                                         start=(j == 0),
                                         stop=(j == KC // 2 - 1),
                                         perf_mode=DR)
                if ob == 0:
                    nc.vector.tensor_copy(ptw[0:8, :], ptp[0:8, :])
                for t in range(TT):
                    close_and_evac(accs[t], t, osl)

            def o_block_touter(ob, w):
                # last o-block: weights long resident; each token tile
                # closes and drains while the next one computes
                osl = slice(512 * ob, 512 * (ob + 1))
                base = 0 if ob % 2 == 0 else 4
                for t in range(TT):
                    acc = psum.tile([128, 512], f32, name=f"acc{t}_{ob}",
                                    tag=f"acc{base + t}")
                    for k in range(KC):
                        nc.tensor.matmul(
                            acc[:],
                            xt[:, k, 128 * t:128 * (t + 1)],
                            w[:, k, :],
                            start=(k == 0), stop=False)
                    if t == TT - 1:
                        close_and_evac(acc, t, osl, final=True)
                    elif t == TT - 2:
                        close_and_evac(acc, t, osl, split_out=True)
                    else:
                        ring = nc.sync if t % 2 == 0 else nc.scalar
                        close_and_evac(acc, t, osl, ring=ring)

            ws = [wt_fetch(0, WG0), wt_fetch(1, WGN)]
            for ob in range(NB - 1):
                if ob + 2 < NB:
                    ws.append(wt_fetch(ob + 2, WGN))
                o_block(ob, ws[ob])
            o_block_touter(NB - 1, ws[NB - 1])

    nc.compile()
    return nc


def _get_nc():
    if "nc" not in _cache:
        _cache["nc"] = _build()
    return _cache["nc"]


def kernel(x, base_weight, lora_A, lora_B, bias, _trace=False, _trace_kwargs=None):
    from concourse.bass_utils import run_bass_kernel_spmd

    nc = _get_nc()

    f16 = np.float16
    x_flat = np.ascontiguousarray(x, dtype=np.float32).reshape(T, D)
    at = np.ascontiguousarray(
        lora_A.T, dtype=np.float32).reshape(KC, 128, 8).transpose(
            1, 0, 2).astype(f16).reshape(128, KC // 2, 2, 8)
    ones = np.ones((1, TC), dtype=f16)

    # xt[p, k, t] = x^T[k*128+p, t]
    xt_shards = [
        np.ascontiguousarray(
            x_flat[TC * i:TC * (i + 1), :]
            .reshape(TC, KC, 128).transpose(2, 1, 0)).astype(f16)
        for i in range(T_SH)
    ]
    # wt[p, ob, k, o] = W[ob*512+o, k*128+p]; replicated across cores
    wt = np.ascontiguousarray(
        np.asarray(base_weight)
        .reshape(NB, 512, KC, 128).transpose(3, 0, 2, 1)).astype(f16)
    bb = np.vstack([2.0 * np.asarray(lora_B).T,
                    np.asarray(bias)[None, :]]).astype(f16)

    in_maps = []
    for c in range(8):
        in_maps.append({
            "xt": xt_shards[c],
            "wt": wt,
            "at": at,
            "bb": bb,
            "ones": ones,
        })

    res = run_bass_kernel_spmd(nc, in_maps, list(range(8)),
                               trace=_trace, **(_trace_kwargs or {}))

    y = np.empty((T, O), dtype=np.float32)
    for c in range(8):
        y[TC * c:TC * (c + 1), :] = res.results[c]["y"]
    out = y.reshape(x.shape[0], x.shape[1], O)
    if _trace:
        return out, res
    return out


# revision 52
# speedup vs baseline: 1.0066x; 1.0066x over previous
"""LoRALinear fused kernel for 8 trn2 NeuronCores.

y = x @ (base + 2*(B@A))^T + bias,  x:[2,2048,4096], base:[4096,4096],
A:[8,4096], B:[4096,8], bias:[4096] -> y:[2,2048,4096], all fp32.

Sharding: 8 token-shards (pure data-parallel, weights replicated).
Per core:
  y_c[512, 4096] = x_c[512,4096] @ W[4096,4096]^T + bias
decomposed as
  y_c = x_c@base^T + [x_c@A^T | 1] @ [2*B^T ; bias].

Token-sharding (vs. tensor-parallel) means each core owns unique
tokens, so the rank-8 PT = A@x^T is computed exactly once per token —
half the PT work of a 4x2 token-x-dout sharding.  Weights stream
just-in-time (33.6MB/core at ~150GB/s sustained, far under the HBM
limit), x^T stays SBUF-resident (4.2MB).

Main/close matmuls run fp16 (11-bit mantissa, same precision class as
the f32r cast path; PSUM accumulates fp32): the PE matmul stream is
the only roofline (~232us busy/core; ~216ns per 512-wide matmul = the
NX issue-rate bound).  PT runs fp8e4m3 DoubleRow (two k-chunks per
instruction), which only perturbs the LoRA term (~11% of y) -> rel
err ~4.3e-3 against the 2e-2 gate.

Schedule notes (each worth real ns on HW):
- All o-blocks are k-outer (consume weight chunks as they arrive);
  accumulator PSUM tags alternate banks 0-3 / 4-7 between o-blocks so
  a block's first matmuls never wait on the previous block's
  evacuation.  The last o-block is t-outer (its weights are long
  resident) so each token tile drains while the next computes and the
  final-drain tail is a single tile.
- All operand layouts are host-packed so every DMA line is >=2KB
  contiguous per partition (keeps HWDGE descriptor counts at 128).
- Every matmul is [K<=128 x 128]x[K x 512]: A^T/ptw/bb are zero-padded
  to full 128 rows/cols on device because switching the PE's active
  row/col-group mask costs ~100ns each way.
- WARMUP dummy matmuls on a zeroed tile run during the initial DMA
  latency window to flip the HAM clock-gate (1.2 -> 2.4GHz) before
  real work starts.
- fp8 copies of x^T are cast on the DVE as chunks arrive (no extra
  HBM traffic).
"""
import sys

sys.path.insert(0, "/opt/trn_rl_repo")

import numpy as np

T_SH = 8                   # token shards (weights replicated)
T, D, O = 4096, 4096, 4096  # flattened tokens, d_in, d_out
TC, OC = T // T_SH, O       # 512 tokens, 4096 outs per core
KC = D // 128              # 32 contraction chunks
NB = OC // 512             # 8 o-blocks of 512 per core
TT = TC // 128             # 4 token tiles per core

# DMA group sizes (in k-chunks) for the weight stream; first o-block
# starts small so the first matmuls' data lands fast.
WG0 = [1, 1, 2, 4, 4, 4, 8, 8]
WGN = [16, 16]
# xt chunk DMA groups (chunk = [128, TC] fp16 = 128KB)
XG = [1, 1, 2, 4, 4, 4, 8, 8]
# Warm-up matmuls run at the cold 1.2GHz clock (~426ns) and flip the HAM
# clock-gate to 2.4GHz after ~3.4us; sized to end right as the first real
# operands land so they don't delay real work.
WARMUP = 24

_cache = {}


def _build():
    import concourse.mybir as mybir
    import concourse.tile as tile
    from concourse import bacc

    f32 = mybir.dt.float32
    f16 = mybir.dt.float16
    f8 = mybir.dt.float8e4
    DR = mybir.MatmulPerfMode.DoubleRow

    nc = bacc.Bacc("TRN2", target_bir_lowering=False, debug=False,
                   num_devices=8)

    # host-packed: xt[p, k, t] = x^T[k*128+p, t] so DMA lines are contiguous
    xt_d = nc.dram_tensor("xt", [128, KC, TC], f16, kind="ExternalInput").ap()
    # weight stream, host-packed so every DMA line is contiguous:
    # wt[p, ob, k, o] = W^T[k*128+p, ob*512+o]
    wt_d = nc.dram_tensor("wt", [128, NB, KC, 512], f16,
                          kind="ExternalInput").ap()
    at_d = nc.dram_tensor("at", [128, KC // 2, 2, 8], f16,
                          kind="ExternalInput").ap()
    # rows 0-7: 2*B^T, row 8: bias  (K=9 close matmul adds lora + bias)
    bb_d = nc.dram_tensor("bb", [9, OC], f16, kind="ExternalInput").ap()
    ones_d = nc.dram_tensor("ones", [1, TC], f16, kind="ExternalInput").ap()
    y_d = nc.dram_tensor("y", [TC, OC], f32, kind="ExternalOutput").ap()

    with tile.TileContext(nc) as tc:
        with (
            tc.tile_pool(name="res", bufs=1) as res,
            tc.tile_pool(name="wres", bufs=2) as wres,
            tc.tile_pool(name="evac", bufs=8) as evac,
            tc.tile_pool(name="x8", bufs=6) as x8p,
            tc.tile_pool(name="psum", bufs=1, space="PSUM") as psum,
        ):
            # PE warm-up: matmuls on a zeroed SBUF tile (no DMA deps) run
            # while the first loads are in flight, so the HAM clock-gate
            # reaches 2.4GHz before real work starts.
            junk = res.tile([128, 512], f16)
            nc.gpsimd.memset(junk[:], 0)
            wacc = psum.tile([128, 512], f32, name="warm", tag="acc4")
            for _ in range(WARMUP):
                nc.tensor.matmul(wacc[:], junk[:, 0:128], junk[:],
                                 start=True, stop=True)
            # A^T zero-padded to 128 columns, fp8, DoubleRow pair layout
            # [p, k-pair, 2, 128]: PT matmuls contract two k-chunks per
            # instruction at half cost, with 128-wide outputs like everything
            # else (switching the PE column mask costs ~100ns each way).
            atp8 = res.tile([128, KC // 2, 2, 128], f8)
            nc.vector.memset(atp8[:], 0)
            # ptw/bb zero-padded to 128 contraction rows for the same reason:
            # a K=9 close matmul flips the row-group mask (+~100ns twice).
            # bb's big memset rides the idle GpSimd so the DVE reaches the
            # fp8 casts in time for the first PT matmul.
            ptw = res.tile([128, TC], f16)
            bb = res.tile([128, OC], f16)
            nc.gpsimd.memset(bb[:], 0)

            # critical-path loads first: at on sync (ahead of weights),
            # xt chunk 0 on scalar, so the first matmuls start ~10us
            at = res.tile([128, KC // 2, 2, 8], f16)
            nc.sync.dma_start(at[:], at_d[:])
            nc.vector.tensor_copy(atp8[:, :, :, 0:8], at[:])
            nc.vector.memset(ptw[:], 0)
            xt = res.tile([128, KC, TC], f16)
            c0 = 0
            for ng in XG:
                nc.scalar.dma_start(xt[:, c0:c0 + ng, :],
                                    xt_d[:, c0:c0 + ng, :])
                c0 += ng
            # close-time tensors ride behind the full xt stream (first
            # close is at ~40us; these only need to land by then)
            nc.scalar.dma_start(bb[0:9, :], bb_d[:])
            # ptw rows 0-7: PT = A@x^T (device), row 8: ones
            nc.scalar.dma_start(ptw[8:9, :], ones_d[:])

            # fp8 copies of the xt chunks for the DoubleRow PT matmuls, cast
            # on the DVE as each chunk arrives (no extra HBM traffic); the
            # rotating pair-buffers pace themselves via WAR deps
            x8tiles = []
            for j in range(KC // 2):
                x8t = x8p.tile([128, 2, TC], f8, name=f"x8_{j}", tag="x8")
                for i in range(2):
                    nc.vector.tensor_copy(x8t[:, i, :], xt[:, 2 * j + i, :])
                x8tiles.append(x8t)

            def wt_fetch(ob, groups):
                w = wres.tile([128, KC, 512], f16, name=f"wtob{ob}",
                              tag="wtob")
                c0 = 0
                for ng in groups:
                    nc.sync.dma_start(w[:, c0:c0 + ng, :],
                                      wt_d[:, ob, c0:c0 + ng, :])
                    c0 += ng
                return w

            def close_and_evac(acc, t, osl, ring=None, split_out=False,
                               final=False):
                ev = evac.tile([128, 512], f32, name=f"ev{t}", tag="ev")
                tsl = slice(128 * t, 128 * (t + 1))
                nc.tensor.matmul(acc[:], ptw[0:128, 128 * t:128 * (t + 1)],
                                 bb[0:128, osl], start=False, stop=True)
                if final:
                    # very last tile: halve the copy across two engines and
                    # the store across both rings to shorten the drain
                    nc.vector.tensor_copy(ev[:, 0:256], acc[:, 0:256])
                    nc.scalar.copy(ev[:, 256:512], acc[:, 256:512])
                else:
                    nc.vector.tensor_copy(ev[:], acc[:])
                if final or split_out:
                    h = slice(osl.start, osl.start + 256)
                    h2 = slice(osl.start + 256, osl.stop)
                    nc.scalar.dma_start(y_d[tsl, h], ev[:, 0:256])
                    nc.sync.dma_start(y_d[tsl, h2], ev[:, 256:512])
                else:
                    (ring or nc.scalar).dma_start(y_d[tsl, osl], ev[:])

            def o_block(ob, w):
                # k-outer: consumes each weight chunk as it arrives; PSUM
                # accumulator banks alternate 0-3/4-7 between o-blocks.
                # o-block 0 interleaves PT = A_pad@x^T (one extra bank).
                osl = slice(512 * ob, 512 * (ob + 1))
                base = 0 if ob % 2 == 0 else 4
                accs = {
                    t: psum.tile([128, 512], f32, name=f"acc{t}_{ob}",
                                 tag=f"acc{base + t}")
                    for t in range(TT)
                }
                if ob == 0:
                    ptp = psum.tile([128, 512], f32, name="ptp", tag="acc4")
                for k in range(KC):
                    for t in range(TT):
                        nc.tensor.matmul(
                            accs[t][:],
                            xt[:, k, 128 * t:128 * (t + 1)],
                            w[:, k, :],
                            start=(k == 0), stop=False)
                    if ob == 0 and k % 2 == 1:
                        j = k // 2
                        nc.tensor.matmul(ptp[:], atp8[:, j, :, :],
                                         x8tiles[j][:],
                                         start=(j == 0),
                                         stop=(j == KC // 2 - 1),
                                         perf_mode=DR)
                if ob == 0:
                    nc.vector.tensor_copy(ptw[0:8, :], ptp[0:8, :])
                for t in range(TT):
                    close_and_evac(accs[t], t, osl)

            def o_block_touter(ob, w):
                # last o-block: weights long resident; each token tile
                # closes and drains while the next one computes
                osl = slice(512 * ob, 512 * (ob + 1))
                base = 0 if ob % 2 == 0 else 4
                for t in range(TT):
                    acc = psum.tile([128, 512], f32, name=f"acc{t}_{ob}",
                                    tag=f"acc{base + t}")
                    for k in range(KC):
                        nc.tensor.matmul(
                            acc[:],
                            xt[:, k, 128 * t:128 * (t + 1)],
                            w[:, k, :],
                            start=(k == 0), stop=False)
                    if t == TT - 1:
                        close_and_evac(acc, t, osl, final=True)
                    elif t == TT - 2:
                        close_and_evac(acc, t, osl, split_out=True)
                    else:
                        ring = nc.sync if t % 2 == 0 else nc.scalar
                        close_and_evac(acc, t, osl, ring=ring)

            ws = [wt_fetch(0, WG0), wt_fetch(1, WGN)]
            for ob in range(NB - 1):
                if ob + 2 < NB:
                    ws.append(wt_fetch(ob + 2, WGN))
                o_block(ob, ws[ob])
            o_block_touter(NB - 1, ws[NB - 1])

    nc.compile()
    return nc


def _get_nc():
    if "nc" not in _cache:
        _cache["nc"] = _build()
    return _cache["nc"]


def kernel(x, base_weight, lora_A, lora_B, bias, _trace=False, _trace_kwargs=None):
    from concourse.bass_utils import run_bass_kernel_spmd

    nc = _get_nc()

    f16 = np.float16
    x_flat = np.ascontiguousarray(x, dtype=np.float32).reshape(T, D)
    at = np.ascontiguousarray(
        lora_A.T, dtype=np.float32).reshape(KC, 128, 8).transpose(
            1, 0, 2).astype(f16).reshape(128, KC // 2, 2, 8)
    ones = np.ones((1, TC), dtype=f16)

    # xt[p, k, t] = x^T[k*128+p, t]
    xt_shards = [
        np.ascontiguousarray(
            x_flat[TC * i:TC * (i + 1), :]
            .reshape(TC, KC, 128).transpose(2, 1, 0)).astype(f16)
        for i in range(T_SH)
    ]
    # wt[p, ob, k, o] = W[ob*512+o, k*128+p]; replicated across cores
    wt = np.ascontiguousarray(
        np.asarray(base_weight)
        .reshape(NB, 512, KC, 128).transpose(3, 0, 2, 1)).astype(f16)
    bb = np.vstack([2.0 * np.asarray(lora_B).T,
                    np.asarray(bias)[None, :]]).astype(f16)

    in_maps = []
    for c in range(8):
        in_maps.append({
            "xt": xt_shards[c],
            "wt": wt,
            "at": at,
            "bb": bb,
            "ones": ones,
        })

    res = run_bass_kernel_spmd(nc, in_maps, list(range(8)),
                               trace=_trace, **(_trace_kwargs or {}))

    y = np.empty((T, O), dtype=np.float32)
    for c in range(8):
        y[TC * c:TC * (c + 1), :] = res.results[c]["y"]
    out = y.reshape(x.shape[0], x.shape[1], O)
    if _trace:
        return out, res
    return out
